# revision 16
# baseline (speedup 1.0000x reference)
"""Trainium2 Bass kernel for nn_CropRoi (3D RoI crop + adaptive max pool).

Contract: kernel(**inputs) takes FULL unsharded inputs
  f:         [B=2, C=128, Df=24, Hf=24, Wf=24] float32 feature map
  inputs:    [B, 1, D=96, H=96, W=96] float32 (only shape used)
  proposals: [N=64, 8] float32 (batch, score, center_zyx, side_zyx)
  scale:     scalar int
and returns the FULL output [N, C, 7, 7, 7] float32.

Design (proposals sharded across 8 cores, 5..10 RoIs per core):
  - (z,y)-cropped slab loads: f[b, :, zlo:zhi, ylo:yhi, :] split across the
    SP and Act HWDGE rings; descriptors of (y_span*96)B hold ~290 GB/s/core
    against a measured ~360 GB/s per-core DMA ceiling.
  - the fp32 slab is converted once per body to a bf16 twin on the Act
    engine (two chunks aligned with the ring split), so every max-pool
    stage runs in the DVE 2x_1p fast mode (0.52 ns/elem vs 1.04).
  - separable adaptive max-pool on DVE with run-batched strided
    instructions. For stages whose window widths mix {d, d+1} (L >= 8) a
    width-uniformized decomposition is used: a width-d base pass over
    maximal constant-delta runs of all 7 bins plus one affine 'extra max'
    pass over the long bins — 25-50%% fewer DVE instructions than plain
    (delta,width)-run chains at identical element count (L=11: 10 -> 5).
    Stage order is optimized per proposal against the calibrated DVE model.
  - width-1 runs are copies: mid-stage copies stay on DVE (a cross-engine
    hop mid-chain stalls DVE's in-order stream); final-stage copies and
    [C,343] convert-packs go to Act, which only feeds the out-DMA.
  - output accumulates in a per-body [C, 10*343] fp32 tile; ONE flat
    out-DMA per body on the GpSimd SWDGE ring (partition-contiguous
    [C, n*343] is ~3x faster than a per-proposal [n,C,343] layout).
    The host re-transposes to [N, C, 7, 7, 7].
  - proposal->core assignment anneals (120k moves with restarts, swaps +
    transfers with variable 5..10 group sizes) a measured per-core lane
    model: max(DMA bytes/290 B/ns, 1.2x modeled DVE ns, Act ns) plus a
    small mean-pressure term so descent continues when the max plateaus.

ONE SPMD program with tc.If per core and a runtime-bounded tc.For_i(reps)
hardware loop for slope timing. All input bytes are re-loaded from DRAM and
all output bytes re-written every iteration (honest steady-state traffic).

Measured: ~18-21 us per exec (machine-noise window) vs 31-33 us baseline;
DMA-only floor for the same traffic ~15.2 us; modeled worst per-core lane
~15.3 us. Relative error 3.1e-03 (bf16 rounding; tolerance 2e-2).
"""

import sys

if "/opt/trn_rl_repo" not in sys.path:
    sys.path.insert(0, "/opt/trn_rl_repo")

import itertools
import os

import numpy as np

BF16SLAB = os.environ.get("KN_BF16SLAB", "1") == "1"   # convert slab on Act
STAGING = os.environ.get("KN_STAGING", "0") == "1"     # body-wide bf16 pack
ODMA = os.environ.get("KN_ODMA", "gps")                # gps | sp | act
INRING = os.environ.get("KN_INRING", "split")          # split | sp
UNROLL = int(os.environ.get("KN_UNROLL", "8"))
NODMA = os.environ.get("KN_NODMA", "0") == "1"    # perf probe: hoist loads, skip oDMA
# Best measured config: BF16SLAB=1 STAGING=0 ODMA=gps INRING=split UNROLL=8

S_OUT = 7
NEG32 = np.float32(np.finfo(np.float32).min)
N_CORES = 8
P_PER_CORE = 8
MAX_REPS = 4096

# measured/derived engine rates (ns per element per partition-line, TRN2)
DVE_1X = 1.0 / 0.96
DVE_2X = 0.5 / 0.96
DVE_OH = 60.0            # SBUF access ~58 cyc
ACT_RATE = 1.0 / 1.2
ACT_OH = 185.0           # 222 cyc @1.2
GPS_RATE = 3.44          # measured: Q7 SW copies run ~2x below the 0.6-eff model
GPS_OH = 160.0           # q7 launch + dispatch
DMA_BNS = 290.0          # effective per-core DMA bytes/ns (measured)


# ----------------------------------------------------------------- host geometry
def _bins_1d(L):
    i = np.arange(S_OUT, dtype=np.int64)
    starts = (i * L) // S_OUT
    ends = -((-(i + 1) * L) // S_OUT)
    widths = np.maximum(ends - starts, 0)
    return [int(v) for v in starts], [int(v) for v in widths]


def build_geometry(f_shape, proposals, scale):
    """Mirror the reference's crop-bound computation exactly (float32 ops)."""
    B, C, Df, Hf, Wf = f_shape
    maxd = np.array([Df, Hf, Wf], np.int32)
    p = np.asarray(proposals, np.float32)
    center = p[:, 2:5].astype(np.float32)
    side = p[:, 5:8].astype(np.float32)
    c0f = center - side / np.float32(2.0)
    c1f = c0f + side
    sc = np.float32(scale)
    c0 = np.floor(c0f / sc).astype(np.int32)
    c1 = np.ceil(c1f / sc).astype(np.int32)
    c0 = np.maximum(c0, 0)
    c1 = np.minimum(c1, maxd[None, :])
    b = np.clip(p[:, 0].astype(np.int32), 0, B - 1)

    geoms = []
    for n in range(p.shape[0]):
        L = (c1[n] - c0[n]).astype(int)
        g = {
            "b": int(b[n]),
            "orig": [int(v) for v in c0[n]],
            "L": [int(v) for v in L],
            "empty": bool((L <= 0).any()),
        }
        g["bins"] = [_bins_1d(L[0]), _bins_1d(L[1]), _bins_1d(L[2])]
        geoms.append(g)
    return geoms


def runs_of(starts, widths):
    """Maximal runs of bins with constant (start-delta, width).
    Returns [(i0, nbins, stride, width)]."""
    runs = []
    i, n = 0, len(starts)
    while i < n:
        w = widths[i]
        j = i + 1
        if j < n and widths[j] == w:
            d = starts[j] - starts[i]
            while j < n and widths[j] == w and starts[j] - starts[j - 1] == d:
                j += 1
        else:
            d = 1
        if j - i == 1:
            d = 1
        runs.append((i, j - i, d, w))
        i = j
    return runs


def stage_ops(starts, widths):
    """Instruction plan for one pooling stage over 7 bins.

    Ops: (kind, i0, nb, bstep, s0, sstep) meaning, for j in [0, nb):
      'new'  dst[i0+j*bstep] = max(src[s0+j*sstep], src[s0+1+j*sstep])
      'acc'  dst[i0+j*bstep] = max(dst[...],        src[s0+j*sstep])
      'copy' dst[i0+j*bstep] = src[s0+j*sstep]

    For min-width d >= 2, widths are in {d, d+1}: emit a width-d base pass
    over maximal constant-delta runs of ALL bins, then one affine 'acc' pass
    per run of long bins (their extra element positions are quasi-affine).
    This cuts instruction count ~40% vs (delta,width)-run chains for the
    alternating-width lengths (L=10,11,12). Element count is unchanged.
    """
    d = min(widths)
    n = len(starts)
    ops = []
    if d == 1:
        for (i0, nb, stride, w) in runs_of(starts, widths):
            if w == 1:
                ops.append(("copy", i0, nb, 1, starts[i0], stride))
            else:
                ops.append(("new", i0, nb, 1, starts[i0], stride))
                for k in range(2, w):
                    ops.append(("acc", i0, nb, 1, starts[i0] + k, stride))
        return ops
    i = 0
    while i < n:
        j = i + 1
        dd = 1
        if j < n:
            dd = starts[j] - starts[i]
            while j < n and starts[j] - starts[j - 1] == dd:
                j += 1
        nb = j - i
        if nb == 1:
            dd = 1
        ops.append(("new", i, nb, 1, starts[i], dd))
        for k in range(2, d):
            ops.append(("acc", i, nb, 1, starts[i] + k, dd))
        i = j
    longs = [i for i in range(n) if widths[i] > d]
    m = 0
    while m < len(longs):
        mm = m + 1
        db = ds = 1
        if mm < len(longs):
            db = longs[mm] - longs[m]
            ds = starts[longs[mm]] - starts[longs[m]]
            while (mm < len(longs) and longs[mm] - longs[mm - 1] == db
                   and starts[longs[mm]] - starts[longs[mm - 1]] == ds):
                mm += 1
        nb = mm - m
        if nb == 1:
            db = ds = 1
        ops.append(("acc", longs[m], nb, db, starts[longs[m]] + d, ds))
        m = mm
    return ops


def plan_proposal(g):
    """Pick the stage order + final-write mode minimizing modeled DVE cost.

    All stages read/write bf16 (the slab is pre-converted per body), so every
    max runs in the DVE 2x_1p fast mode except x-axis runs with stride 2 or
    single-bin runs (innermost AP not packed).
    """
    if g["empty"]:
        return {"order": [], "est": {"dve": 0.0, "act": 300.0, "gps": 0.0}}
    L = list(g["L"])
    todo = [a for a in range(3) if L[a] != S_OUT]
    best = None
    for perm in itertools.permutations(todo):
        ext = list(L)
        dve = 0.0
        for si, a in enumerate(perm):
            oth = 1
            for d in range(3):
                if d != a:
                    oth *= ext[d]
            starts, widths = g["bins"][a]
            for (kind, i0, nb, bstep, s0, sstep) in stage_ops(starts, widths):
                fd = nb * oth
                fast = not (a == 2 and (bstep != 1 or sstep != 1
                                        or nb == 1))
                if kind == "copy":
                    dve += DVE_OH + fd * 0.3        # DVE 4x bf16 copy
                else:
                    dve += DVE_OH + fd * (DVE_2X if fast else DVE_1X)
            ext[a] = S_OUT
        if best is None or dve < best[0]:
            best = (dve, list(perm))
    dve, order = best
    return {"order": order, "est": {"dve": dve, "act": 0.0, "gps": 0.0}}


# ----------------------------------------------------------------- assignment
COMP_FUDGE = 1.2         # measured HW / modeled-DVE ratio
P_MAX = 10


def assign_cores(geoms, f_shape):
    """Partition proposals into 8 groups (sizes 5..P_MAX) minimizing the
    worst per-core lane: max(DMA ns, fudged DVE ns, Act ns).
    Slabs are (z,y)-joint bounding boxes per batch."""
    B, C, Df, Hf, Wf = f_shape
    n = len(geoms)
    plans = [plan_proposal(g) for g in geoms]
    dve = [p["est"]["dve"] for p in plans]

    def slabs_of(idxs):
        sl = {}
        for i in idxs:
            g = geoms[i]
            if g["empty"]:
                continue
            zlo, zhi = g["orig"][0], g["orig"][0] + g["L"][0]
            ylo, yhi = g["orig"][1], g["orig"][1] + g["L"][1]
            b = g["b"]
            if b in sl:
                a = sl[b]
                sl[b] = (min(a[0], zlo), max(a[1], zhi),
                         min(a[2], ylo), max(a[3], yhi))
            else:
                sl[b] = (zlo, zhi, ylo, yhi)
        return sl

    def core_cost(idxs):
        sl = slabs_of(idxs)
        slab_el = sum((zhi - zlo) * (yhi - ylo) * Wf
                      for zlo, zhi, ylo, yhi in sl.values())
        by = slab_el * 4 * C + len(idxs) * C * S_OUT ** 3 * 4
        dma = by / DMA_BNS + len(sl) * 1200.0
        comp = COMP_FUDGE * sum(dve[i] for i in idxs) + 800.0
        act = (slab_el * ACT_RATE + 2 * ACT_OH * len(sl)
               + len(idxs) * (ACT_OH + 343 * ACT_RATE) + 700.0)
        # hard SBUF cap: single slab tile <= 26 KB/partition (z*y*96B)
        for zlo, zhi, ylo, yhi in sl.values():
            if (zhi - zlo) * (yhi - ylo) * Wf * 4 > 26624:
                dma += 1e6
        return max(dma, comp, act), by

    def score(cores):
        worst = 0.0
        tot = 0.0
        csum = 0.0
        for idxs in cores:
            if not (5 <= len(idxs) <= P_MAX):
                return 1e9
            c, by = core_cost(idxs)
            worst = max(worst, c)
            csum += c
            tot += by
        # secondary mean term keeps descent pressure when the max plateaus
        return max(worst, tot / 2750.0) + 0.03 * csum / 8.0

    order = sorted(range(n), key=lambda i: (geoms[i]["b"],
                                            geoms[i]["orig"][0],
                                            geoms[i]["orig"][1]))
    base = n // N_CORES
    cores = [order[k * base:(k + 1) * base] for k in range(N_CORES)]
    for j, i in enumerate(order[base * N_CORES:]):
        cores[j].append(i)

    import random
    rnd = random.Random(0)
    cur = score(cores)
    best = cur
    best_cores = [list(c) for c in cores]
    for it in range(120000):
        if it % 40000 == 39999:
            # restart from best
            cores = [list(c) for c in best_cores]
            cur = best
        a, b2 = rnd.randrange(N_CORES), rnd.randrange(N_CORES)
        if a == b2:
            continue
        if rnd.random() < 0.5 and len(cores[a]) > 5 and len(cores[b2]) < P_MAX:
            # transfer one proposal a -> b2
            ia = rnd.randrange(len(cores[a]))
            item = cores[a].pop(ia)
            cores[b2].append(item)
            s = score(cores)
            if s <= cur:
                cur = s
            else:
                cores[b2].pop()
                cores[a].insert(ia, item)
        else:
            ia = rnd.randrange(len(cores[a]))
            ib = rnd.randrange(len(cores[b2]))
            cores[a][ia], cores[b2][ib] = cores[b2][ib], cores[a][ia]
            s = score(cores)
            if s <= cur:
                cur = s
            else:
                cores[a][ia], cores[b2][ib] = cores[b2][ib], cores[a][ia]
        if cur < best:
            best = cur
            best_cores = [list(c) for c in cores]
    return [{"idxs": c, "slabs": slabs_of(c)} for c in best_cores]


# ----------------------------------------------------------------- bass program
class EngineRouter:
    """Greedy per-core accumulated-load router for copies."""

    def __init__(self, nc):
        self.nc = nc
        self.load = {"dve": 0.0, "act": 0.0, "gps": 0.0}

    def charge(self, eng, ns):
        self.load[eng] += ns

    def copy(self, dst, src, fd, src_fp32, allow_dve=True):
        """Emit dst=src copy.

        Mid-stage copies (allow_dve=True) stay on DVE: a cross-engine hop in
        the middle of a stage chain stalls DVE's in-order stream for far more
        than the 4x-mode copy costs. Final packs (allow_dve=False) feed only
        the out-DMA, so they go to Act."""
        nc = self.nc
        if allow_dve:
            self.load["dve"] += DVE_OH + fd * (DVE_2X if src_fp32 else 0.3)
            nc.vector.tensor_copy(dst, src)
        else:
            self.load["act"] += ACT_OH + fd * ACT_RATE
            nc.scalar.copy(dst, src)


def _emit_pool(nc, mybir, wpool, g, plan, src, src_fp32, offs0, dst_final,
               p, router):
    """Emit the pooling pipeline for one proposal.

    src: slab tile (bf16 or fp32 per src_fp32), offs0: crop offset inside it.
    dst_final: callable(p) -> ([C,343] 2D AP, [C,7,7,7] 4D AP) for the final
    destination (bf16 staging slice or a fresh per-proposal tf tile)."""
    S = S_OUT
    C = 128

    if g["empty"]:
        return              # fp32 NEG memset happens post-pack in caller

    fin2, fin4 = dst_final(p)
    L = list(g["L"])
    order = plan["order"]
    ext = list(L)
    offs = list(offs0)
    cur = src
    cur_fp32 = src_fp32

    if not order:
        # all dims exactly 7: strided pack straight out of the slab (Act:
        # feeds only the Act pack, so no DVE round-trip)
        sidx = tuple([slice(None)] + [slice(offs[d], offs[d] + S)
                                      for d in range(3)])
        router.copy(fin4, cur[sidx], 343, cur_fp32, allow_dve=False)
        return

    for si, a in enumerate(order):
        last = si == len(order) - 1
        starts, widths = g["bins"][a]
        shape = [C] + [S if d == a else ext[d] for d in range(3)]
        if last:
            newt = None
            dst4 = fin4
        else:
            newt = wpool.tile(shape, mybir.dt.bfloat16, tag=f"t{si}")
            dst4 = newt
        oth = 1
        for d in range(3):
            if d != a:
                oth *= shape[1 + d]
        for (kind, i0, nb, bstep, s0, sstep) in stage_ops(starts, widths):
            fd = nb * oth

            def didx():
                idx = [slice(None)] * 4
                for d in range(3):
                    idx[1 + d] = slice(0, shape[1 + d])
                idx[1 + a] = slice(i0, i0 + (nb - 1) * bstep + 1, bstep)
                return tuple(idx)

            def sidx(extra):
                idx = [slice(None)] * 4
                for d in range(3):
                    if d == a:
                        b0 = offs[a] + s0 + extra
                        idx[1 + d] = slice(b0, b0 + (nb - 1) * sstep + 1,
                                           sstep)
                    else:
                        idx[1 + d] = slice(offs[d], offs[d] + shape[1 + d])
                return tuple(idx)

            dst = dst4[didx()]
            if kind == "copy":
                # final-stage copies feed only the Act pack -> Act; mid-stage
                # copies stay on DVE (cross-engine hop would stall the chain)
                router.copy(dst, cur[sidx(0)], fd, cur_fp32,
                            allow_dve=not last)
            else:
                fast = ((not cur_fp32)
                        and not (a == 2 and (bstep != 1 or sstep != 1
                                             or nb == 1)))
                rate = DVE_2X if fast else DVE_1X
                if kind == "new":
                    nc.vector.tensor_max(dst, cur[sidx(0)], cur[sidx(1)])
                else:
                    nc.vector.tensor_max(dst, dst, cur[sidx(0)])
                router.charge("dve", DVE_OH + fd * rate)
        cur = newt
        cur_fp32 = False
        offs = [0, 0, 0]
        ext[a] = S


def _emit_slab_loads(nc, mybir, fpool, core, f_ap):
    """Issue y-cropped z-slab DMA(s), each split across SP+Act rings.
    Returns {batch: (tile, zlo, ylo, zm)}."""
    C = 128
    Wf = f_ap.shape[4]
    fh = {}
    for b, (zlo, zhi, ylo, yhi) in core["slabs"].items():
        zs, ys = zhi - zlo, yhi - ylo
        ft = fpool.tile([C, zs, ys, Wf], mybir.dt.float32, tag="fh")
        zm = zlo + (zs + 1) // 2
        if INRING == "sp":
            nc.sync.dma_start(out=ft[:], in_=f_ap[b, :, zlo:zhi, ylo:yhi, :])
        else:
            nc.sync.dma_start(out=ft[:, 0:zm - zlo],
                              in_=f_ap[b, :, zlo:zm, ylo:yhi, :])
            if zm < zhi:
                nc.scalar.dma_start(out=ft[:, zm - zlo:],
                                    in_=f_ap[b, :, zm:zhi, ylo:yhi, :])
        fh[b] = (ft, zlo, ylo, zm)
    return fh


def _emit_slab_convert(nc, mybir, bpool, fh, router):
    """Convert each fp32 slab to a bf16 twin on Act, two z-chunks aligned
    with the DMA ring split (each chunk depends on one in-DMA only).
    With BF16SLAB off, pass the fp32 slabs through unchanged."""
    bh = {}
    for b, (ft, zlo, ylo, zm) in fh.items():
        if not BF16SLAB:
            bh[b] = (ft, zlo, ylo, True)
            continue
        C, zs, ys, Wf = ft.shape
        bt = bpool.tile([C, zs, ys, Wf], mybir.dt.bfloat16, tag="bh")
        zc = zm - zlo
        for (z0, z1) in ((0, zc), (zc, zs)):
            if z1 <= z0:
                continue
            fd = (z1 - z0) * ys * Wf
            router.charge("act", ACT_OH + fd * ACT_RATE)
            nc.scalar.copy(bt[:, z0:z1], ft[:, z0:z1])
        bh[b] = (bt, zlo, ylo, False)
    return bh


def _emit_core_body(nc, mybir, wpool, opool, spool, core, geoms, plans, bh,
                    o_ap, router):
    C = 128
    S = S_OUT
    n_k = len(core["idxs"])
    obig = opool.tile([C, P_MAX * 343], mybir.dt.float32, tag="obig")
    if STAGING:
        stg = spool.tile([C, P_MAX * 343], mybir.dt.bfloat16, tag="stg")

        def dst_final(p):
            f2 = stg[:, p * 343:(p + 1) * 343]
            return f2, f2.rearrange("c (a b d) -> c a b d", a=S, b=S, d=S)
    else:
        def dst_final(p):
            tf = wpool.tile([C, S, S, S], mybir.dt.bfloat16, tag="tf")
            return tf[:].rearrange("c a b d -> c (a b d)"), tf[:]

    finals = {}
    empties = []
    for j, i in enumerate(core["idxs"]):
        g = geoms[i]
        if g["empty"]:
            empties.append(j)
            continue
        bt, zlo, ylo, src_fp32 = bh[g["b"]]
        offs0 = (g["orig"][0] - zlo, g["orig"][1] - ylo, g["orig"][2])
        if STAGING:
            df = dst_final
        else:
            cache = {}

            def df(p, cache=cache):
                if p not in cache:
                    cache[p] = dst_final(p)
                return cache[p]
        _emit_pool(nc, mybir, wpool, g, plans[i], bt, src_fp32, offs0, df,
                   j, router)
        if not STAGING:
            finals[j] = cache[j]
    if STAGING:
        nc.scalar.copy(obig[:, :n_k * 343], stg[:, :n_k * 343])
        router.charge("act", ACT_OH + n_k * 343 * ACT_RATE)
    else:
        for j, (f2, f4) in finals.items():
            nc.scalar.copy(obig[:, j * 343:(j + 1) * 343], f2)
            router.charge("act", ACT_OH + 343 * ACT_RATE)
    for j in empties:
        # exact fp32 NEG fill (bf16 would round float32.min to -inf)
        nc.vector.memset(obig[:, j * 343:(j + 1) * 343], float(NEG32))
        router.charge("dve", DVE_OH + 343 * DVE_2X)
    if not NODMA:
        eng = {"gps": nc.gpsimd, "sp": nc.sync, "act": nc.scalar}[ODMA]
        eng.dma_start(out=o_ap[:, :n_k * 343], in_=obig[:, :n_k * 343])


def build_program(f_shape, geoms, plans, cores):
    import concourse.bacc as bacc
    import concourse.tile as tile
    import concourse.mybir as mybir

    B, C, Df, Hf, Wf = f_shape
    assert C == 128
    nc = bacc.Bacc("TRN2", target_bir_lowering=False, debug=False,
                   num_devices=1)
    f_ap = nc.dram_tensor("f", [B, C, Df, Hf, Wf], mybir.dt.float32,
                          kind="ExternalInput").ap()
    reps_t = nc.dram_tensor("reps", [1, 1], mybir.dt.uint32,
                            kind="ExternalInput")
    cid_t = nc.dram_tensor("cid", [1, 1], mybir.dt.uint32,
                           kind="ExternalInput")
    o_ap = nc.dram_tensor("o", [C, P_MAX * 343],
                          mybir.dt.float32, kind="ExternalOutput").ap()

    with tile.TileContext(nc) as tc:
        rtmp = nc.alloc_registers("reps_reg", mybir.ALL_ENGINES)
        nc.regs_load(rtmp, reps_t[0:1, 0:1])
        rv = nc.snap(rtmp, donate=True, min_val=1, max_val=MAX_REPS)
        ctmp = nc.alloc_registers("cid_reg", mybir.ALL_ENGINES)
        nc.regs_load(ctmp, cid_t[0:1, 0:1])
        cid = nc.snap(ctmp, donate=True, min_val=0, max_val=N_CORES - 1)
        with tc.tile_pool(name="fpool", bufs=int(os.environ.get("KN_FBUFS", "3"))) as fpool, \
             tc.tile_pool(name="bpool", bufs=4) as bpool, \
             tc.tile_pool(name="wpool", bufs=3) as wpool, \
             tc.tile_pool(name="spool", bufs=3) as spool, \
             tc.tile_pool(name="opool", bufs=3) as opool:
            for k in range(N_CORES):
                with tc.If(cid == k):
                    if NODMA:
                        router = EngineRouter(nc)
                        fh0 = _emit_slab_loads(nc, mybir, fpool,
                                               cores[k], f_ap)
                        bh0 = _emit_slab_convert(nc, mybir, bpool,
                                                 fh0, router)
                        with tc.For_i(0, rv):
                            for _u in range(UNROLL):
                                _emit_core_body(nc, mybir, wpool, opool,
                                                spool, cores[k], geoms,
                                                plans, bh0, o_ap,
                                                EngineRouter(nc))
                    else:
                        with tc.For_i(0, rv):
                            router = EngineRouter(nc)
                            fh_cur = _emit_slab_loads(nc, mybir, fpool,
                                                      cores[k], f_ap)
                            bh_cur = _emit_slab_convert(nc, mybir, bpool,
                                                        fh_cur, router)
                            for _u in range(UNROLL):
                                router.charge("gps", 1100.0)   # SWDGE oDMA
                                if _u + 1 < UNROLL:
                                    fh_next = _emit_slab_loads(
                                        nc, mybir, fpool, cores[k], f_ap)
                                    bh_next = _emit_slab_convert(
                                        nc, mybir, bpool, fh_next, router)
                                _emit_core_body(nc, mybir, wpool, opool,
                                                spool, cores[k], geoms,
                                                plans, bh_cur, o_ap, router)
                                if _u + 1 < UNROLL:
                                    fh_cur = fh_next
                                    bh_cur = bh_next
    nc.compile()
    return nc


# ----------------------------------------------------------------- entry points
def make_fast_runner(nc, f, ncores=N_CORES):
    """Low-jitter benchmark runner (same as v1)."""
    import jax
    import jax.numpy as jnp
    from jax.sharding import Mesh, PartitionSpec, NamedSharding
    from jax.experimental.shard_map import shard_map
    import concourse.mybir as mybir
    from concourse.bass2jax import (_bass_exec_p, install_neuronx_cc_hook,
                                    partition_id_tensor)

    install_neuronx_cc_hook()
    partition_name = (nc.partition_id_tensor.name
                      if nc.partition_id_tensor else None)
    in_names, out_names, out_avals = [], [], []
    for alloc in nc.m.functions[0].allocations:
        if not isinstance(alloc, mybir.MemoryLocationSet):
            continue
        name = alloc.memorylocations[0].name
        if alloc.kind == "ExternalInput":
            if name != partition_name:
                in_names.append(name)
        elif alloc.kind == "ExternalOutput":
            out_names.append(name)
            out_avals.append(jax.core.ShapedArray(
                tuple(alloc.tensor_shape), mybir.dt.np(alloc.dtype)))
    n_params = len(in_names)
    all_names = tuple(in_names + out_names +
                      ([partition_name] if partition_name else []))

    def _body(*args):
        operands = list(args)
        if partition_name is not None:
            operands.append(partition_id_tensor())
        outs = _bass_exec_p.bind(
            *operands,
            out_avals=tuple(out_avals),
            in_names=all_names,
            out_names=tuple(out_names),
            lowering_input_output_aliases=(),
            sim_require_finite=True,
            sim_require_nnan=True,
            nc=nc,
        )
        return tuple(outs)

    devices = jax.devices()[:ncores]
    mesh = Mesh(np.asarray(devices), ("core",))
    n_outs = len(out_names)
    sharded = jax.jit(
        shard_map(_body, mesh=mesh,
                  in_specs=(PartitionSpec("core"),) * (n_params + n_outs),
                  out_specs=(PartitionSpec("core"),) * n_outs,
                  check_rep=False),
        donate_argnums=tuple(range(n_params, n_params + n_outs)),
        keep_unused=True,
    )
    sh = NamedSharding(mesh, PartitionSpec("core"))
    oshape = (ncores * 128, P_MAX * 343)
    zeros_fn = jax.jit(lambda: jnp.zeros(oshape, jnp.float32),
                       out_shardings=sh)
    f_dev = jax.device_put(
        np.concatenate([f] * ncores, axis=0), sh)
    cid_dev = jax.device_put(
        np.arange(ncores, dtype=np.uint32).reshape(ncores, 1), sh)

    def run(reps):
        reps_arr = jax.device_put(
            np.full((ncores, 1), reps, np.uint32), sh)
        outs = sharded(f_dev, reps_arr, cid_dev, zeros_fn())
        outs[0].block_until_ready()
        return outs

    return run


def run_program(nc, f, reps=1):
    from concourse.bass_utils import run_bass_kernel_spmd

    in_maps = [
        {"f": f, "reps": np.array([[reps]], np.uint32),
         "cid": np.array([[k]], np.uint32)}
        for k in range(N_CORES)
    ]
    res = run_bass_kernel_spmd(nc, in_maps, core_ids=list(range(N_CORES)))
    return res


def kernel(**inputs):
    f = np.ascontiguousarray(np.asarray(inputs["f"], dtype=np.float32))
    proposals = np.asarray(inputs["proposals"], dtype=np.float32)
    scale = int(np.asarray(inputs["scale"]))
    geoms = build_geometry(f.shape, proposals, scale)
    plans = [plan_proposal(g) for g in geoms]
    cores = assign_cores(geoms, f.shape)
    nc = build_program(f.shape, geoms, plans, cores)
    kernel.last_nc = nc
    kernel.last_f = f
    res = run_program(nc, f, reps=1)
    out = np.empty((len(geoms), 128, S_OUT, S_OUT, S_OUT), np.float32)
    for k in range(N_CORES):
        part = np.asarray(res.results[k]["o"])          # [128, P_MAX*343]
        part = part.reshape(128, P_MAX, S_OUT, S_OUT, S_OUT)
        for j, i in enumerate(cores[k]["idxs"]):
            out[i] = part[:, j]
    return out


kernel.last_nc = None
kernel.last_f = None


# revision 18
# speedup vs baseline: 1.1236x; 1.1236x over previous
"""Trainium2 Bass kernel for nn_CropRoi (3D RoI crop + adaptive max pool).

Contract: kernel(**inputs) takes FULL unsharded inputs
  f:         [B=2, C=128, Df=24, Hf=24, Wf=24] float32 feature map
  inputs:    [B, 1, D=96, H=96, W=96] float32 (only shape used)
  proposals: [N=64, 8] float32 (batch, score, center_zyx, side_zyx)
  scale:     scalar int
and returns the FULL output [N, C, 7, 7, 7] float32.

Design (proposals sharded across 8 cores, 5..10 RoIs per core):
  - (z,y)-cropped slab loads: f[b, :, zlo:zhi, ylo:yhi, :] split across the
    SP and Act HWDGE rings; descriptors of (y_span*96)B hold ~290 GB/s/core
    against a measured ~360 GB/s per-core DMA ceiling.
  - the fp32 slab is converted once per body to a bf16 twin on the Act
    engine (two chunks aligned with the ring split), so every max-pool
    stage runs in the DVE 2x_1p fast mode (0.52 ns/elem vs 1.04).
  - separable adaptive max-pool on DVE with run-batched strided
    instructions. For stages whose window widths mix {d, d+1} (L >= 8) a
    width-uniformized decomposition is used: a width-d base pass over
    maximal constant-delta runs of all 7 bins plus one affine 'extra max'
    pass over the long bins — 25-50%% fewer DVE instructions than plain
    (delta,width)-run chains at identical element count (L=11: 10 -> 5).
    Stage order is optimized per proposal against the calibrated DVE model.
  - width-1 runs are copies: mid-stage copies stay on DVE (a cross-engine
    hop mid-chain stalls DVE's in-order stream); final-stage copies and
    [C,343] convert-packs go to Act, which only feeds the out-DMA.
  - output accumulates in a per-body [C, 10*343] fp32 tile; ONE flat
    out-DMA per body on the GpSimd SWDGE ring (partition-contiguous
    [C, n*343] is ~3x faster than a per-proposal [n,C,343] layout).
    The host re-transposes to [N, C, 7, 7, 7].
  - proposal->core assignment anneals (120k moves with restarts, swaps +
    transfers with variable 5..10 group sizes) a measured per-core lane
    model: max(DMA bytes/290 B/ns, 1.2x modeled DVE ns, Act ns) plus a
    small mean-pressure term so descent continues when the max plateaus.

ONE SPMD program with tc.If per core and a runtime-bounded tc.For_i(reps)
hardware loop for slope timing. All input bytes are re-loaded from DRAM and
all output bytes re-written every iteration (honest steady-state traffic).

Measured: ~18-21 us per exec (machine-noise window) vs 31-33 us baseline;
DMA-only floor for the same traffic ~15.2 us; modeled worst per-core lane
~15.3 us. Relative error 3.1e-03 (bf16 rounding; tolerance 2e-2).
"""

import sys

if "/opt/trn_rl_repo" not in sys.path:
    sys.path.insert(0, "/opt/trn_rl_repo")

import itertools
import os

import numpy as np

BF16SLAB = os.environ.get("KN_BF16SLAB", "1") == "1"   # convert slab on Act
STAGING = os.environ.get("KN_STAGING", "0") == "1"     # body-wide bf16 pack
ODMA = os.environ.get("KN_ODMA", "gps")                # gps | sp | act
INRING = os.environ.get("KN_INRING", "split")          # split | sp
UNROLL = int(os.environ.get("KN_UNROLL", "8"))
NODMA = os.environ.get("KN_NODMA", "0") == "1"    # perf probe: hoist loads, skip oDMA
# Best measured config: BF16SLAB=1 STAGING=0 ODMA=gps INRING=split UNROLL=8

S_OUT = 7
NEG32 = np.float32(np.finfo(np.float32).min)
N_CORES = 8
P_PER_CORE = 8
MAX_REPS = 4096

# measured/derived engine rates (ns per element per partition-line, TRN2)
DVE_1X = 1.0 / 0.96
DVE_2X = 0.5 / 0.96
DVE_OH = 60.0            # SBUF access ~58 cyc
ACT_RATE = 1.0 / 1.2
ACT_OH = 185.0           # 222 cyc @1.2
GPS_RATE = 3.44          # measured: Q7 SW copies run ~2x below the 0.6-eff model
GPS_OH = 160.0           # q7 launch + dispatch
DMA_BNS = 290.0          # effective per-core DMA bytes/ns (measured)


# ----------------------------------------------------------------- host geometry
def _bins_1d(L):
    i = np.arange(S_OUT, dtype=np.int64)
    starts = (i * L) // S_OUT
    ends = -((-(i + 1) * L) // S_OUT)
    widths = np.maximum(ends - starts, 0)
    return [int(v) for v in starts], [int(v) for v in widths]


def build_geometry(f_shape, proposals, scale):
    """Mirror the reference's crop-bound computation exactly (float32 ops)."""
    B, C, Df, Hf, Wf = f_shape
    maxd = np.array([Df, Hf, Wf], np.int32)
    p = np.asarray(proposals, np.float32)
    center = p[:, 2:5].astype(np.float32)
    side = p[:, 5:8].astype(np.float32)
    c0f = center - side / np.float32(2.0)
    c1f = c0f + side
    sc = np.float32(scale)
    c0 = np.floor(c0f / sc).astype(np.int32)
    c1 = np.ceil(c1f / sc).astype(np.int32)
    c0 = np.maximum(c0, 0)
    c1 = np.minimum(c1, maxd[None, :])
    b = np.clip(p[:, 0].astype(np.int32), 0, B - 1)

    geoms = []
    for n in range(p.shape[0]):
        L = (c1[n] - c0[n]).astype(int)
        g = {
            "b": int(b[n]),
            "orig": [int(v) for v in c0[n]],
            "L": [int(v) for v in L],
            "empty": bool((L <= 0).any()),
        }
        g["bins"] = [_bins_1d(L[0]), _bins_1d(L[1]), _bins_1d(L[2])]
        geoms.append(g)
    return geoms


def runs_of(starts, widths):
    """Maximal runs of bins with constant (start-delta, width).
    Returns [(i0, nbins, stride, width)]."""
    runs = []
    i, n = 0, len(starts)
    while i < n:
        w = widths[i]
        j = i + 1
        if j < n and widths[j] == w:
            d = starts[j] - starts[i]
            while j < n and widths[j] == w and starts[j] - starts[j - 1] == d:
                j += 1
        else:
            d = 1
        if j - i == 1:
            d = 1
        runs.append((i, j - i, d, w))
        i = j
    return runs


def stage_ops(starts, widths):
    """Instruction plan for one pooling stage over 7 bins.

    Ops: (kind, i0, nb, bstep, s0, sstep) meaning, for j in [0, nb):
      'new'  dst[i0+j*bstep] = max(src[s0+j*sstep], src[s0+1+j*sstep])
      'acc'  dst[i0+j*bstep] = max(dst[...],        src[s0+j*sstep])
      'copy' dst[i0+j*bstep] = src[s0+j*sstep]

    For min-width d >= 2, widths are in {d, d+1}: emit a width-d base pass
    over maximal constant-delta runs of ALL bins, then one affine 'acc' pass
    per run of long bins (their extra element positions are quasi-affine).
    This cuts instruction count ~40% vs (delta,width)-run chains for the
    alternating-width lengths (L=10,11,12). Element count is unchanged.
    """
    d = min(widths)
    n = len(starts)
    ops = []
    if d == 1:
        for (i0, nb, stride, w) in runs_of(starts, widths):
            if w == 1:
                ops.append(("copy", i0, nb, 1, starts[i0], stride))
            else:
                ops.append(("new", i0, nb, 1, starts[i0], stride))
                for k in range(2, w):
                    ops.append(("acc", i0, nb, 1, starts[i0] + k, stride))
        return ops
    i = 0
    while i < n:
        j = i + 1
        dd = 1
        if j < n:
            dd = starts[j] - starts[i]
            while j < n and starts[j] - starts[j - 1] == dd:
                j += 1
        nb = j - i
        if nb == 1:
            dd = 1
        ops.append(("new", i, nb, 1, starts[i], dd))
        for k in range(2, d):
            ops.append(("acc", i, nb, 1, starts[i] + k, dd))
        i = j
    longs = [i for i in range(n) if widths[i] > d]
    m = 0
    while m < len(longs):
        mm = m + 1
        db = ds = 1
        if mm < len(longs):
            db = longs[mm] - longs[m]
            ds = starts[longs[mm]] - starts[longs[m]]
            while (mm < len(longs) and longs[mm] - longs[mm - 1] == db
                   and starts[longs[mm]] - starts[longs[mm - 1]] == ds):
                mm += 1
        nb = mm - m
        if nb == 1:
            db = ds = 1
        ops.append(("acc", longs[m], nb, db, starts[longs[m]] + d, ds))
        m = mm
    return ops


def plan_proposal(g):
    """Pick the stage order + final-write mode minimizing modeled DVE cost.

    All stages read/write bf16 (the slab is pre-converted per body), so every
    max runs in the DVE 2x_1p fast mode except x-axis runs with stride 2 or
    single-bin runs (innermost AP not packed).
    """
    if g["empty"]:
        return {"order": [], "est": {"dve": 0.0, "act": 300.0, "gps": 0.0}}
    L = list(g["L"])
    todo = [a for a in range(3) if L[a] != S_OUT]
    best = None
    for perm in itertools.permutations(todo):
        ext = list(L)
        dve = 0.0
        for si, a in enumerate(perm):
            oth = 1
            for d in range(3):
                if d != a:
                    oth *= ext[d]
            starts, widths = g["bins"][a]
            for (kind, i0, nb, bstep, s0, sstep) in stage_ops(starts, widths):
                fd = nb * oth
                fast = not (a == 2 and (bstep != 1 or sstep != 1
                                        or nb == 1))
                if kind == "copy":
                    dve += DVE_OH + fd * 0.3        # DVE 4x bf16 copy
                else:
                    dve += DVE_OH + fd * (DVE_2X if fast else DVE_1X)
            ext[a] = S_OUT
        if best is None or dve < best[0]:
            best = (dve, list(perm))
    dve, order = best
    return {"order": order, "est": {"dve": dve, "act": 0.0, "gps": 0.0}}


# ----------------------------------------------------------------- assignment
COMP_FUDGE = 1.2         # measured HW / modeled-DVE ratio
P_MAX = 10


def assign_cores(geoms, f_shape):
    """Partition proposals into 8 groups (sizes 5..P_MAX) minimizing the
    worst per-core lane: max(DMA ns, fudged DVE ns, Act ns).
    Slabs are (z,y)-joint bounding boxes per batch."""
    B, C, Df, Hf, Wf = f_shape
    n = len(geoms)
    plans = [plan_proposal(g) for g in geoms]
    dve = [p["est"]["dve"] for p in plans]

    def slabs_of(idxs):
        sl = {}
        for i in idxs:
            g = geoms[i]
            if g["empty"]:
                continue
            zlo, zhi = g["orig"][0], g["orig"][0] + g["L"][0]
            ylo, yhi = g["orig"][1], g["orig"][1] + g["L"][1]
            b = g["b"]
            if b in sl:
                a = sl[b]
                sl[b] = (min(a[0], zlo), max(a[1], zhi),
                         min(a[2], ylo), max(a[3], yhi))
            else:
                sl[b] = (zlo, zhi, ylo, yhi)
        return sl

    def core_cost(idxs):
        sl = slabs_of(idxs)
        slab_el = sum((zhi - zlo) * (yhi - ylo) * Wf
                      for zlo, zhi, ylo, yhi in sl.values())
        by = slab_el * 4 * C + len(idxs) * C * S_OUT ** 3 * 4
        dma = by / DMA_BNS + len(sl) * 1200.0
        comp = COMP_FUDGE * sum(dve[i] for i in idxs) + 800.0
        act = (slab_el * ACT_RATE + 2 * ACT_OH * len(sl)
               + len(idxs) * (ACT_OH + 343 * ACT_RATE) + 700.0)
        # hard SBUF cap: single slab tile <= 26 KB/partition (z*y*96B)
        for zlo, zhi, ylo, yhi in sl.values():
            if (zhi - zlo) * (yhi - ylo) * Wf * 4 > 26624:
                dma += 1e6
        return max(dma, comp, act), by

    def score(cores):
        worst = 0.0
        tot = 0.0
        csum = 0.0
        for idxs in cores:
            if not (5 <= len(idxs) <= P_MAX):
                return 1e9
            c, by = core_cost(idxs)
            worst = max(worst, c)
            csum += c
            tot += by
        # secondary mean term keeps descent pressure when the max plateaus
        return max(worst, tot / 2750.0) + 0.03 * csum / 8.0

    order = sorted(range(n), key=lambda i: (geoms[i]["b"],
                                            geoms[i]["orig"][0],
                                            geoms[i]["orig"][1]))
    base = n // N_CORES
    start_cores = [order[k * base:(k + 1) * base] for k in range(N_CORES)]
    for j, i in enumerate(order[base * N_CORES:]):
        start_cores[j].append(i)

    import random
    best = None
    best_cores = None
    for seed in (0, 1, 2):
        rnd = random.Random(seed)
        cores = [list(c) for c in start_cores]
        cur = score(cores)
        if best is None or cur < best:
            best = cur
            best_cores = [list(c) for c in cores]
        for it in range(120000):
            if it % 40000 == 39999:
                cores = [list(c) for c in best_cores]
                cur = best
            a, b2 = rnd.randrange(N_CORES), rnd.randrange(N_CORES)
            if a == b2:
                continue
            if (rnd.random() < 0.5 and len(cores[a]) > 5
                    and len(cores[b2]) < P_MAX):
                # transfer one proposal a -> b2
                ia = rnd.randrange(len(cores[a]))
                item = cores[a].pop(ia)
                cores[b2].append(item)
                sc_ = score(cores)
                if sc_ <= cur:
                    cur = sc_
                else:
                    cores[b2].pop()
                    cores[a].insert(ia, item)
            else:
                ia = rnd.randrange(len(cores[a]))
                ib = rnd.randrange(len(cores[b2]))
                cores[a][ia], cores[b2][ib] = cores[b2][ib], cores[a][ia]
                sc_ = score(cores)
                if sc_ <= cur:
                    cur = sc_
                else:
                    cores[a][ia], cores[b2][ib] = cores[b2][ib], cores[a][ia]
            if cur < best:
                best = cur
                best_cores = [list(c) for c in cores]
    return [{"idxs": c, "slabs": slabs_of(c)} for c in best_cores]


# ----------------------------------------------------------------- bass program
class EngineRouter:
    """Greedy per-core accumulated-load router for copies."""

    def __init__(self, nc):
        self.nc = nc
        self.load = {"dve": 0.0, "act": 0.0, "gps": 0.0}

    def charge(self, eng, ns):
        self.load[eng] += ns

    def copy(self, dst, src, fd, src_fp32, allow_dve=True):
        """Emit dst=src copy.

        Mid-stage copies (allow_dve=True) stay on DVE: a cross-engine hop in
        the middle of a stage chain stalls DVE's in-order stream for far more
        than the 4x-mode copy costs. Final packs (allow_dve=False) feed only
        the out-DMA, so they go to Act."""
        nc = self.nc
        if allow_dve:
            self.load["dve"] += DVE_OH + fd * (DVE_2X if src_fp32 else 0.3)
            nc.vector.tensor_copy(dst, src)
        else:
            self.load["act"] += ACT_OH + fd * ACT_RATE
            nc.scalar.copy(dst, src)


def _emit_pool(nc, mybir, wpool, g, plan, src, src_fp32, offs0, dst_final,
               p, router):
    """Emit the pooling pipeline for one proposal.

    src: slab tile (bf16 or fp32 per src_fp32), offs0: crop offset inside it.
    dst_final: callable(p) -> ([C,343] 2D AP, [C,7,7,7] 4D AP) for the final
    destination (bf16 staging slice or a fresh per-proposal tf tile)."""
    S = S_OUT
    C = 128

    if g["empty"]:
        return              # fp32 NEG memset happens post-pack in caller

    fin2, fin4 = dst_final(p)
    L = list(g["L"])
    order = plan["order"]
    ext = list(L)
    offs = list(offs0)
    cur = src
    cur_fp32 = src_fp32

    if not order:
        # all dims exactly 7: strided pack straight out of the slab (Act:
        # feeds only the Act pack, so no DVE round-trip)
        sidx = tuple([slice(None)] + [slice(offs[d], offs[d] + S)
                                      for d in range(3)])
        router.copy(fin4, cur[sidx], 343, cur_fp32, allow_dve=False)
        return

    for si, a in enumerate(order):
        last = si == len(order) - 1
        starts, widths = g["bins"][a]
        shape = [C] + [S if d == a else ext[d] for d in range(3)]
        if last:
            newt = None
            dst4 = fin4
        else:
            newt = wpool.tile(shape, mybir.dt.bfloat16, tag=f"t{si}")
            dst4 = newt
        oth = 1
        for d in range(3):
            if d != a:
                oth *= shape[1 + d]
        for (kind, i0, nb, bstep, s0, sstep) in stage_ops(starts, widths):
            fd = nb * oth

            def didx():
                idx = [slice(None)] * 4
                for d in range(3):
                    idx[1 + d] = slice(0, shape[1 + d])
                idx[1 + a] = slice(i0, i0 + (nb - 1) * bstep + 1, bstep)
                return tuple(idx)

            def sidx(extra):
                idx = [slice(None)] * 4
                for d in range(3):
                    if d == a:
                        b0 = offs[a] + s0 + extra
                        idx[1 + d] = slice(b0, b0 + (nb - 1) * sstep + 1,
                                           sstep)
                    else:
                        idx[1 + d] = slice(offs[d], offs[d] + shape[1 + d])
                return tuple(idx)

            dst = dst4[didx()]
            if kind == "copy":
                # final-stage copies feed only the Act pack -> Act; mid-stage
                # copies stay on DVE (cross-engine hop would stall the chain)
                router.copy(dst, cur[sidx(0)], fd, cur_fp32,
                            allow_dve=not last)
            else:
                fast = ((not cur_fp32)
                        and not (a == 2 and (bstep != 1 or sstep != 1
                                             or nb == 1)))
                rate = DVE_2X if fast else DVE_1X
                if kind == "new":
                    nc.vector.tensor_max(dst, cur[sidx(0)], cur[sidx(1)])
                else:
                    nc.vector.tensor_max(dst, dst, cur[sidx(0)])
                router.charge("dve", DVE_OH + fd * rate)
        cur = newt
        cur_fp32 = False
        offs = [0, 0, 0]
        ext[a] = S


def _emit_slab_loads(nc, mybir, fpool, core, f_ap):
    """Issue y-cropped z-slab DMA(s), each split across SP+Act rings.
    Returns {batch: (tile, zlo, ylo, zm)}."""
    C = 128
    Wf = f_ap.shape[4]
    fh = {}
    for b, (zlo, zhi, ylo, yhi) in core["slabs"].items():
        zs, ys = zhi - zlo, yhi - ylo
        ft = fpool.tile([C, zs, ys, Wf], mybir.dt.float32, tag="fh")
        zm = zlo + (zs + 1) // 2
        if INRING == "sp":
            nc.sync.dma_start(out=ft[:], in_=f_ap[b, :, zlo:zhi, ylo:yhi, :])
        else:
            nc.sync.dma_start(out=ft[:, 0:zm - zlo],
                              in_=f_ap[b, :, zlo:zm, ylo:yhi, :])
            if zm < zhi:
                nc.scalar.dma_start(out=ft[:, zm - zlo:],
                                    in_=f_ap[b, :, zm:zhi, ylo:yhi, :])
        fh[b] = (ft, zlo, ylo, zm)
    return fh


def _emit_slab_convert(nc, mybir, bpool, fh, router):
    """Convert each fp32 slab to a bf16 twin on Act, two z-chunks aligned
    with the DMA ring split (each chunk depends on one in-DMA only).
    With BF16SLAB off, pass the fp32 slabs through unchanged."""
    bh = {}
    for b, (ft, zlo, ylo, zm) in fh.items():
        if not BF16SLAB:
            bh[b] = (ft, zlo, ylo, True)
            continue
        C, zs, ys, Wf = ft.shape
        bt = bpool.tile([C, zs, ys, Wf], mybir.dt.bfloat16, tag="bh")
        zc = zm - zlo
        for (z0, z1) in ((0, zc), (zc, zs)):
            if z1 <= z0:
                continue
            fd = (z1 - z0) * ys * Wf
            router.charge("act", ACT_OH + fd * ACT_RATE)
            nc.scalar.copy(bt[:, z0:z1], ft[:, z0:z1])
        bh[b] = (bt, zlo, ylo, False)
    return bh


def _emit_core_body(nc, mybir, wpool, opool, spool, core, geoms, plans, bh,
                    o_ap, router):
    C = 128
    S = S_OUT
    n_k = len(core["idxs"])
    obig = opool.tile([C, P_MAX * 343], mybir.dt.float32, tag="obig")
    if STAGING:
        stg = spool.tile([C, P_MAX * 343], mybir.dt.bfloat16, tag="stg")

        def dst_final(p):
            f2 = stg[:, p * 343:(p + 1) * 343]
            return f2, f2.rearrange("c (a b d) -> c a b d", a=S, b=S, d=S)
    else:
        def dst_final(p):
            tf = wpool.tile([C, S, S, S], mybir.dt.bfloat16, tag="tf")
            return tf[:].rearrange("c a b d -> c (a b d)"), tf[:]

    finals = {}
    empties = []
    for j, i in enumerate(core["idxs"]):
        g = geoms[i]
        if g["empty"]:
            empties.append(j)
            continue
        bt, zlo, ylo, src_fp32 = bh[g["b"]]
        offs0 = (g["orig"][0] - zlo, g["orig"][1] - ylo, g["orig"][2])
        if STAGING:
            df = dst_final
        else:
            cache = {}

            def df(p, cache=cache):
                if p not in cache:
                    cache[p] = dst_final(p)
                return cache[p]
        _emit_pool(nc, mybir, wpool, g, plans[i], bt, src_fp32, offs0, df,
                   j, router)
        if not STAGING:
            finals[j] = cache[j]
    if STAGING:
        nc.scalar.copy(obig[:, :n_k * 343], stg[:, :n_k * 343])
        router.charge("act", ACT_OH + n_k * 343 * ACT_RATE)
    else:
        for j, (f2, f4) in finals.items():
            nc.scalar.copy(obig[:, j * 343:(j + 1) * 343], f2)
            router.charge("act", ACT_OH + 343 * ACT_RATE)
    for j in empties:
        # exact fp32 NEG fill (bf16 would round float32.min to -inf)
        nc.vector.memset(obig[:, j * 343:(j + 1) * 343], float(NEG32))
        router.charge("dve", DVE_OH + 343 * DVE_2X)
    if not NODMA:
        eng = {"gps": nc.gpsimd, "sp": nc.sync, "act": nc.scalar}[ODMA]
        eng.dma_start(out=o_ap[:, :n_k * 343], in_=obig[:, :n_k * 343])


def build_program(f_shape, geoms, plans, cores):
    import concourse.bacc as bacc
    import concourse.tile as tile
    import concourse.mybir as mybir

    B, C, Df, Hf, Wf = f_shape
    assert C == 128
    nc = bacc.Bacc("TRN2", target_bir_lowering=False, debug=False,
                   num_devices=1)
    f_ap = nc.dram_tensor("f", [B, C, Df, Hf, Wf], mybir.dt.float32,
                          kind="ExternalInput").ap()
    reps_t = nc.dram_tensor("reps", [1, 1], mybir.dt.uint32,
                            kind="ExternalInput")
    cid_t = nc.dram_tensor("cid", [1, 1], mybir.dt.uint32,
                           kind="ExternalInput")
    o_ap = nc.dram_tensor("o", [C, P_MAX * 343],
                          mybir.dt.float32, kind="ExternalOutput").ap()

    with tile.TileContext(nc) as tc:
        rtmp = nc.alloc_registers("reps_reg", mybir.ALL_ENGINES)
        nc.regs_load(rtmp, reps_t[0:1, 0:1])
        rv = nc.snap(rtmp, donate=True, min_val=1, max_val=MAX_REPS)
        ctmp = nc.alloc_registers("cid_reg", mybir.ALL_ENGINES)
        nc.regs_load(ctmp, cid_t[0:1, 0:1])
        cid = nc.snap(ctmp, donate=True, min_val=0, max_val=N_CORES - 1)
        with tc.tile_pool(name="fpool", bufs=int(os.environ.get("KN_FBUFS", "3"))) as fpool, \
             tc.tile_pool(name="bpool", bufs=4) as bpool, \
             tc.tile_pool(name="wpool", bufs=3) as wpool, \
             tc.tile_pool(name="spool", bufs=3) as spool, \
             tc.tile_pool(name="opool", bufs=3) as opool:
            for k in range(N_CORES):
                with tc.If(cid == k):
                    if NODMA:
                        router = EngineRouter(nc)
                        fh0 = _emit_slab_loads(nc, mybir, fpool,
                                               cores[k], f_ap)
                        bh0 = _emit_slab_convert(nc, mybir, bpool,
                                                 fh0, router)
                        with tc.For_i(0, rv):
                            for _u in range(UNROLL):
                                _emit_core_body(nc, mybir, wpool, opool,
                                                spool, cores[k], geoms,
                                                plans, bh0, o_ap,
                                                EngineRouter(nc))
                    else:
                        with tc.For_i(0, rv):
                            router = EngineRouter(nc)
                            fh_cur = _emit_slab_loads(nc, mybir, fpool,
                                                      cores[k], f_ap)
                            bh_cur = _emit_slab_convert(nc, mybir, bpool,
                                                        fh_cur, router)
                            for _u in range(UNROLL):
                                router.charge("gps", 1100.0)   # SWDGE oDMA
                                if _u + 1 < UNROLL:
                                    fh_next = _emit_slab_loads(
                                        nc, mybir, fpool, cores[k], f_ap)
                                    bh_next = _emit_slab_convert(
                                        nc, mybir, bpool, fh_next, router)
                                _emit_core_body(nc, mybir, wpool, opool,
                                                spool, cores[k], geoms,
                                                plans, bh_cur, o_ap, router)
                                if _u + 1 < UNROLL:
                                    fh_cur = fh_next
                                    bh_cur = bh_next
    nc.compile()
    return nc


# ----------------------------------------------------------------- entry points
def make_fast_runner(nc, f, ncores=N_CORES):
    """Low-jitter benchmark runner (same as v1)."""
    import jax
    import jax.numpy as jnp
    from jax.sharding import Mesh, PartitionSpec, NamedSharding
    from jax.experimental.shard_map import shard_map
    import concourse.mybir as mybir
    from concourse.bass2jax import (_bass_exec_p, install_neuronx_cc_hook,
                                    partition_id_tensor)

    install_neuronx_cc_hook()
    partition_name = (nc.partition_id_tensor.name
                      if nc.partition_id_tensor else None)
    in_names, out_names, out_avals = [], [], []
    for alloc in nc.m.functions[0].allocations:
        if not isinstance(alloc, mybir.MemoryLocationSet):
            continue
        name = alloc.memorylocations[0].name
        if alloc.kind == "ExternalInput":
            if name != partition_name:
                in_names.append(name)
        elif alloc.kind == "ExternalOutput":
            out_names.append(name)
            out_avals.append(jax.core.ShapedArray(
                tuple(alloc.tensor_shape), mybir.dt.np(alloc.dtype)))
    n_params = len(in_names)
    all_names = tuple(in_names + out_names +
                      ([partition_name] if partition_name else []))

    def _body(*args):
        operands = list(args)
        if partition_name is not None:
            operands.append(partition_id_tensor())
        outs = _bass_exec_p.bind(
            *operands,
            out_avals=tuple(out_avals),
            in_names=all_names,
            out_names=tuple(out_names),
            lowering_input_output_aliases=(),
            sim_require_finite=True,
            sim_require_nnan=True,
            nc=nc,
        )
        return tuple(outs)

    devices = jax.devices()[:ncores]
    mesh = Mesh(np.asarray(devices), ("core",))
    n_outs = len(out_names)
    sharded = jax.jit(
        shard_map(_body, mesh=mesh,
                  in_specs=(PartitionSpec("core"),) * (n_params + n_outs),
                  out_specs=(PartitionSpec("core"),) * n_outs,
                  check_rep=False),
        donate_argnums=tuple(range(n_params, n_params + n_outs)),
        keep_unused=True,
    )
    sh = NamedSharding(mesh, PartitionSpec("core"))
    oshape = (ncores * 128, P_MAX * 343)
    zeros_fn = jax.jit(lambda: jnp.zeros(oshape, jnp.float32),
                       out_shardings=sh)
    f_dev = jax.device_put(
        np.concatenate([f] * ncores, axis=0), sh)
    cid_dev = jax.device_put(
        np.arange(ncores, dtype=np.uint32).reshape(ncores, 1), sh)

    def run(reps):
        reps_arr = jax.device_put(
            np.full((ncores, 1), reps, np.uint32), sh)
        outs = sharded(f_dev, reps_arr, cid_dev, zeros_fn())
        outs[0].block_until_ready()
        return outs

    return run


def run_program(nc, f, reps=1):
    from concourse.bass_utils import run_bass_kernel_spmd

    in_maps = [
        {"f": f, "reps": np.array([[reps]], np.uint32),
         "cid": np.array([[k]], np.uint32)}
        for k in range(N_CORES)
    ]
    res = run_bass_kernel_spmd(nc, in_maps, core_ids=list(range(N_CORES)))
    return res


def kernel(**inputs):
    f = np.ascontiguousarray(np.asarray(inputs["f"], dtype=np.float32))
    proposals = np.asarray(inputs["proposals"], dtype=np.float32)
    scale = int(np.asarray(inputs["scale"]))
    geoms = build_geometry(f.shape, proposals, scale)
    plans = [plan_proposal(g) for g in geoms]
    cores = assign_cores(geoms, f.shape)
    nc = build_program(f.shape, geoms, plans, cores)
    kernel.last_nc = nc
    kernel.last_f = f
    res = run_program(nc, f, reps=1)
    out = np.empty((len(geoms), 128, S_OUT, S_OUT, S_OUT), np.float32)
    for k in range(N_CORES):
        part = np.asarray(res.results[k]["o"])          # [128, P_MAX*343]
        part = part.reshape(128, P_MAX, S_OUT, S_OUT, S_OUT)
        for j, i in enumerate(cores[k]["idxs"]):
            out[i] = part[:, j]
    return out


kernel.last_nc = None
kernel.last_f = None
